# revision 5
# baseline (speedup 1.0000x reference)
"""Trainium2 Bass kernel for GCN message passing (nn_GCN_38628935860365).

out = PReLU( segment_sum( adj_vals * (x @ W^T + b)[adj_cols], adj_rows ), alpha )

v2 strategy (8 NeuronCores, SPMD, full inputs in / full output out):
  - Destination-node sharding: core c owns dest rows [c*12500, (c+1)*12500).
  - Phase A: h_c = x_c @ W^T + b on the tensor engine (fp16, fp32 PSUM),
    in 4 node-chunks of [3200,3200,3200,2944] local rows.
  - Phase B: after each chunk k, a chunked AllGather moves chunk k of all
    cores into h2 (chunk-major layout, each chunk <= 25600 rows so int16
    gather indices can address it).  Overlaps with phase A + early phase C.
  - Phase C: per supertile of T=4 dest tiles x 4 source chunks, ONE fused
    gpsimd dma_gather call (contiguous edge list, minimal padding) pulls
    the per-edge source rows of h2 into SBUF.  The scatter one-hot S for
    ALL the call's (chunk, tile) work items is built in TWO fused DVE
    tensor_tensor ops using stride-0 broadcast APs, then the tensor
    engine accumulates S^T.T @ msgs into per-tile PSUM.  Work items are
    the UNION across cores of (chunk, tile) pairs so the SPMD graph is
    common; per-core dest/val columns mask non-participating rows to 0.
  - PReLU epilogue on the scalar engine, fp16 DMA out, host upcasts.
"""
import math
import sys
import types

import numpy as np

N_NODES = 100000
N_FEATURES = 512
N_HIDDEN = 256
N_EDGES = 3200000
N_CORES = 8
ALPHA = 0.25

SHARD = N_NODES // N_CORES          # 12500
P_NODES = 12544                     # padded to 128
N_TILES = P_NODES // 128            # 98
T_SUPER = 4                         # dest tiles per supertile
N_ST = (N_TILES + T_SUPER - 1) // T_SUPER   # 25
CHUNK_LOC = [640, 3968, 3968, 3968]         # local rows per source chunk
CHUNK_TILE_END = [5, 36, 67, 98]            # phase-A tile idx ending each chunk
ROFF2 = [0, 5120, 36864, 68608]             # h2 row offset per chunk
RLEN2 = [5120, 31744, 31744, 31744]
N_SEGS = N_ST * 4

_CACHE = {}
TRACE = False
LAST_EXEC_NS = None


def _install_ntff_shim():
    """Make bass_utils' optional trace path importable (harmless if unused)."""
    if "antenv.axon_hooks" in sys.modules:
        return
    mod = types.ModuleType("antenv.axon_hooks")
    mod._hook = None
    mod.set_axon_ntff_profile_hook = lambda h: setattr(mod, "_hook", h)
    mod.get_axon_ntff_profile_hook = lambda: mod._hook
    sys.modules["antenv.axon_hooks"] = mod
    try:
        from trn_agent_boot.trn_boot import _ntff_profile_via_ctypes
        hook = _ntff_profile_via_ctypes("/opt/axon/libaxon_pjrt.so")
        if hook is not None:
            mod.set_axon_ntff_profile_hook(hook)
    except Exception:
        pass


def _reset_device():
    try:
        import ctypes
        import jax
        jax.devices()
        ctypes.CDLL("/opt/axon/libaxon_pjrt.so").axon_reset()
    except Exception:
        pass


def _preprocess(x, adj_rows, adj_cols, adj_vals, W, b):
    xf = x[0]
    core_of = adj_rows // SHARD

    per_core = []
    for c in range(N_CORES):
        m = core_of == c
        rl = adj_rows[m] - c * SHARD
        t = rl >> 7
        dl = (rl & 127).astype(np.float32)
        s = adj_cols[m]
        sc, sl = np.divmod(s, SHARD)
        cb = np.array([0, 640, 4608, 8576, 12544])
        k = (np.searchsorted(cb, sl, side="right") - 1).astype(np.int64)
        szk = np.array(CHUNK_LOC)[k]
        i16 = (sc * szk + (sl - cb[k])).astype(np.int16)
        st = t // T_SUPER
        perm = np.lexsort((i16, t, k, st))
        seg_id = (st * 4 + k)[perm]
        per_core.append(dict(seg_id=seg_id, t=t[perm], dl=dl[perm],
                             i16=i16[perm], val=adj_vals[m][perm].astype(np.float32)))

    # common padded segment sizes (multiple of 128, max across cores)
    seg_pad = np.zeros(N_SEGS, np.int64)
    for c in range(N_CORES):
        bc = np.bincount(per_core[c]["seg_id"], minlength=N_SEGS)
        seg_pad = np.maximum(seg_pad, bc)
    seg_pad = ((seg_pad + 127) // 128) * 128
    seg_off = np.concatenate([[0], np.cumsum(seg_pad)])   # edge offsets
    ni_total = int(seg_off[-1])
    seg_chunks = seg_pad // 128
    chunk_base = np.concatenate([[0], np.cumsum(seg_chunks)])
    nc_total = int(chunk_base[-1])

    # per-core position of each edge inside its padded segment
    all_keys = []
    core_pos = []
    for c in range(N_CORES):
        cc = per_core[c]
        sid = cc["seg_id"]
        # rank within segment (edges already sorted by seg_id)
        start = np.searchsorted(sid, np.arange(N_SEGS))
        pos = np.arange(len(sid)) - start[sid]
        core_pos.append(pos)
        chunk_g = chunk_base[sid] + (pos >> 7)
        all_keys.append(chunk_g * N_TILES + cc["t"])

    item_keys = np.unique(np.concatenate(all_keys))
    n_items = len(item_keys)
    item_chunk = item_keys // N_TILES
    item_tile = (item_keys % N_TILES).astype(np.int64)
    # segment of each item (via chunk)
    item_seg = np.searchsorted(chunk_base, item_chunk, side="right") - 1
    item_st = item_seg // 4
    # start/stop flags: first/last item of (st, tile)
    gkey = item_st * N_TILES + item_tile
    _, first_idx = np.unique(gkey, return_index=True)
    item_start = np.zeros(n_items, bool)
    item_start[first_idx] = True
    last_idx = n_items - 1 - np.unique(gkey[::-1], return_index=True)[1]
    item_stop = np.zeros(n_items, bool)
    item_stop[last_idx] = True
    item_chunk_local = item_chunk - chunk_base[item_seg]
    seg_item_base = np.searchsorted(item_seg, np.arange(N_SEGS))
    seg_item_end = np.searchsorted(item_seg, np.arange(N_SEGS), side="right")

    in_maps = []
    for c in range(N_CORES):
        cc = per_core[c]
        pos = core_pos[c]
        gpos = seg_off[cc["seg_id"]] + pos          # global padded edge slot
        idx_flat = np.zeros(ni_total, np.int16)
        idx_flat[gpos] = cc["i16"]
        idx_w = np.tile(idx_flat.reshape(-1, 16).T, (8, 1))

        item_idx = np.searchsorted(item_keys, all_keys[c])
        dest_arr = np.zeros((128, n_items), np.float16)
        val_arr = np.zeros((128, n_items), np.float16)
        dest_arr[gpos & 127, item_idx] = cc["dl"]
        val_arr[gpos & 127, item_idx] = cc["val"]

        xs = np.zeros((P_NODES, N_FEATURES), np.float32)
        xs[:SHARD] = xf[c * SHARD: (c + 1) * SHARD]
        in_maps.append({
            "xT": np.ascontiguousarray(xs.T).astype(np.float16),
            "wT": np.ascontiguousarray(W.T).astype(np.float16),
            "bias": np.asarray(b, np.float32).astype(np.float16).reshape(1, N_HIDDEN),
            "idx": np.ascontiguousarray(idx_w),
            "dest": np.ascontiguousarray(dest_arr),
            "val": np.ascontiguousarray(val_arr),
        })

    meta = dict(
        ni_total=ni_total, n_items=n_items,
        seg_pad=seg_pad, seg_off=seg_off, seg_chunks=seg_chunks,
        cmax=int(seg_chunks.max()),
        imax=int((seg_item_end - seg_item_base).max()),
        seg_item_base=seg_item_base, seg_item_end=seg_item_end,
        item_chunk_local=item_chunk_local, item_tile=item_tile,
        item_start=item_start, item_stop=item_stop,
    )
    return in_maps, meta


def _build_kernel(meta):
    from concourse import bacc, mybir
    import concourse.tile as tile

    F16, F32 = mybir.dt.float16, mybir.dt.float32
    I16, I32 = mybir.dt.int16, mybir.dt.int32
    F, HID = N_FEATURES, N_HIDDEN
    K_TILES = F // 128
    CMAX, IMAX = meta["cmax"], meta["imax"]
    seg_pad, seg_off = meta["seg_pad"], meta["seg_off"]
    seg_ib, seg_ie = meta["seg_item_base"], meta["seg_item_end"]
    it_cl, it_t = meta["item_chunk_local"], meta["item_tile"]
    it_sta, it_sto = meta["item_start"], meta["item_stop"]

    nc = bacc.Bacc(None, target_bir_lowering=False, num_devices=N_CORES,
                   num_swdge_queues=4)
    xT_t = nc.dram_tensor("xT", [F, P_NODES], F16, kind="ExternalInput")
    wT_t = nc.dram_tensor("wT", [F, HID], F16, kind="ExternalInput")
    bias_t = nc.dram_tensor("bias", [1, HID], F16, kind="ExternalInput")
    idx_t = nc.dram_tensor("idx", [128, meta["ni_total"] // 16], I16,
                           kind="ExternalInput")
    dest_t = nc.dram_tensor("dest", [128, meta["n_items"]], F16,
                            kind="ExternalInput")
    val_t = nc.dram_tensor("val", [128, meta["n_items"]], F16,
                           kind="ExternalInput")
    out_t = nc.dram_tensor("out", [P_NODES, HID], F16, kind="ExternalOutput")

    h_own = nc.dram_tensor("h_own", [P_NODES, HID], F16)
    h2 = nc.dram_tensor("h2", [N_CORES * P_NODES, HID], F16, addr_space="Shared")

    with tile.TileContext(nc) as tc:
        with (
            tc.tile_pool(name="const", bufs=1) as cpool,
            tc.tile_pool(name="xsl", bufs=3) as xpool,
            tc.tile_pool(name="hsb", bufs=3) as hpool,
            tc.tile_pool(name="idxp", bufs=6) as ipool,
            tc.tile_pool(name="msgs", bufs=5) as mpool,
            tc.tile_pool(name="st", bufs=3) as spool,
            tc.tile_pool(name="outp", bufs=3) as opool,
            tc.tile_pool(name="psA", bufs=2, space="PSUM") as ppA,
            tc.tile_pool(name="psC", bufs=6, space="PSUM") as ppC,
        ):
            iota_i32 = cpool.tile([128, 128], I32)
            nc.gpsimd.iota(iota_i32[:], pattern=[[1, 128]], base=0,
                           channel_multiplier=0)
            iota_f16 = cpool.tile([128, 128], F16)
            nc.vector.tensor_copy(iota_f16[:], iota_i32[:])

            wt_sb = cpool.tile([128, K_TILES, HID], F16)
            for kt in range(K_TILES):
                nc.sync.dma_start(out=wt_sb[:, kt, :],
                                  in_=wT_t[kt * 128: (kt + 1) * 128, :])
            ones_sb = cpool.tile([1, 128], F16)
            nc.vector.memset(ones_sb[:], 1.0)
            bias_sb = cpool.tile([1, HID], F16)
            nc.sync.dma_start(out=bias_sb[:], in_=bias_t[:, :])
            dest_sb = cpool.tile([128, meta["n_items"]], F16)
            nc.sync.dma_start(out=dest_sb[:], in_=dest_t[:, :])
            val_sb = cpool.tile([128, meta["n_items"]], F16)
            nc.sync.dma_start(out=val_sb[:], in_=val_t[:, :])

            # ---- phase A + chunked AllGather ----
            SLAB = 512
            boundary = {CHUNK_TILE_END[k] - 1: k for k in range(4)}
            for sl in range(math.ceil(P_NODES / SLAB)):
                w = min(SLAB, P_NODES - sl * SLAB)
                xsl = xpool.tile([128, K_TILES, SLAB], F16, tag="xsl")
                for kt in range(K_TILES):
                    nc.sync.dma_start(
                        out=xsl[:, kt, :w],
                        in_=xT_t[kt * 128: (kt + 1) * 128,
                                 sl * SLAB: sl * SLAB + w])
                for j in range(w // 128):
                    jt = sl * (SLAB // 128) + j       # global 128-row tile
                    psum_h = ppA.tile([128, HID], F32, space="PSUM", tag="psA")
                    for kt in range(K_TILES):
                        nc.tensor.matmul(
                            psum_h[:], lhsT=xsl[:, kt, j * 128: (j + 1) * 128],
                            rhs=wt_sb[:, kt, :], start=(kt == 0), stop=False)
                    nc.tensor.matmul(psum_h[:], lhsT=ones_sb[:], rhs=bias_sb[:],
                                     start=False, stop=True)
                    h_sb = hpool.tile([128, HID], F16, tag="hsb")
                    nc.scalar.activation(h_sb[:], psum_h[:],
                                         func=mybir.ActivationFunctionType.Copy)
                    r0 = jt * 128
                    nc.sync.dma_start(out=h_own[r0: r0 + 128, :], in_=h_sb[:])
                    if jt in boundary:
                        k = boundary[jt]
                        lo = sum(CHUNK_LOC[:k])
                        nc.gpsimd.collective_compute(
                            "AllGather", mybir.AluOpType.bypass,
                            replica_groups=[list(range(N_CORES))],
                            ins=[h_own[lo: lo + CHUNK_LOC[k], :].opt()],
                            outs=[h2[ROFF2[k]: ROFF2[k] + RLEN2[k], :].opt()],
                        )

            # ---- phase C ----
            g = 0
            psum_of = {}
            for st in range(N_ST):
                for r in range(4):
                    seg = st * 4 + r
                    pad = int(seg_pad[seg])
                    if pad == 0:
                        continue
                    nch = pad // 128
                    col0 = int(seg_off[seg]) // 16
                    idx_sb = ipool.tile([128, (CMAX * 128) // 16], I16, tag="idx")
                    nc.sync.dma_start(out=idx_sb[:, : pad // 16],
                                      in_=idx_t[:, col0: col0 + pad // 16])
                    msgs = mpool.tile([128, CMAX, HID], F16, tag="msgs")
                    nc.gpsimd.dma_gather(
                        out_ap=msgs[:, :nch, :],
                        in_ap=h2[ROFF2[r]: ROFF2[r] + RLEN2[r], :],
                        idxs_ap=idx_sb[:, : pad // 16],
                        num_idxs=pad,
                        num_idxs_reg=pad,
                        elem_size=HID,
                        single_packet=False,
                        queue_num=g % 4,
                    )
                    g += 1
                    i0, i1 = int(seg_ib[seg]), int(seg_ie[seg])
                    ni = i1 - i0
                    if ni == 0:
                        continue
                    s_t = spool.tile([128, IMAX, 128], F16, tag="st")
                    nc.vector.tensor_tensor(
                        s_t[:, :ni, :],
                        iota_f16[:, :].unsqueeze(1).broadcast_to([128, ni, 128]),
                        dest_sb[:, i0:i1].unsqueeze(2).broadcast_to([128, ni, 128]),
                        op=mybir.AluOpType.is_equal)
                    nc.vector.tensor_tensor(
                        s_t[:, :ni, :], s_t[:, :ni, :],
                        val_sb[:, i0:i1].unsqueeze(2).broadcast_to([128, ni, 128]),
                        op=mybir.AluOpType.mult)
                    for j in range(ni):
                        i = i0 + j
                        t = int(it_t[i])
                        if it_sta[i]:
                            psum_of[t] = ppC.tile([128, HID], F32, space="PSUM",
                                                  tag="psC", name="psum_c")
                        nc.tensor.matmul(
                            psum_of[t][:], lhsT=s_t[:, j, :],
                            rhs=msgs[:, int(it_cl[i]), :],
                            start=bool(it_sta[i]), stop=bool(it_sto[i]))
                        if it_sto[i]:
                            out_sb = opool.tile([128, HID], F16, tag="out")
                            nc.scalar.activation(
                                out_sb[:], psum_of[t][:],
                                func=mybir.ActivationFunctionType.Prelu,
                                alpha=ALPHA)
                            nc.sync.dma_start(
                                out=out_t[t * 128: (t + 1) * 128, :],
                                in_=out_sb[:])
                            del psum_of[t]
    nc.finalize()
    return nc


def kernel(x, adj_rows, adj_cols, adj_vals, W, b, alpha):
    x = np.asarray(x, np.float32)
    adj_rows = np.asarray(adj_rows, np.int64)
    adj_cols = np.asarray(adj_cols, np.int64)
    adj_vals = np.asarray(adj_vals, np.float32)
    W = np.asarray(W, np.float32)
    b = np.asarray(b, np.float32)

    _install_ntff_shim()
    _reset_device()
    from concourse.bass_utils import run_bass_kernel_spmd

    in_maps, meta = _preprocess(x, adj_rows, adj_cols, adj_vals, W, b)
    key = ("gcn_v2", meta["ni_total"], meta["n_items"])
    if key not in _CACHE:
        _CACHE[key] = _build_kernel(meta)
    nc = _CACHE[key]
    global LAST_EXEC_NS
    res = run_bass_kernel_spmd(nc, in_maps, core_ids=list(range(N_CORES)),
                               trace=TRACE)
    LAST_EXEC_NS = res.exec_time_ns

    out = np.empty((1, N_NODES, N_HIDDEN), np.float32)
    for c in range(N_CORES):
        oc = res.results[c]["out"]
        out[0, c * SHARD: (c + 1) * SHARD] = oc[:SHARD].astype(np.float32)
    return out
